# revision 1
# baseline (speedup 1.0000x reference)
"""Trainium2 Bass kernel for nn_LocalSmoother (LN -> QKV -> RoPE -> 32-token
block-diagonal attention -> out-proj -> residual).

Sharding: B*L = 16384 tokens split evenly across 8 cores (2048 tokens each,
64 chunks of 32). Attention is block-diagonal over 32-token chunks, so shards
are fully independent (pure SPMD, no collectives). Weights are replicated.

Per-core layout strategy:
  - LayerNorm in token-partition layout (bn_stats/bn_aggr), normalize via one
    fused tensor_scalar, output fp16.
  - xn transposed to feature-partition layout (XT) via DMA xbar transpose.
  - QKV as fp16 PE matmuls producing q^T/k^T (feature-partition) and V
    (token-partition).
  - RoPE: qc = q * cos fused into the PSUM->SBUF move; rotate-half is a
    +-32-partition shuffle done with SBUF->SBUF DMA; sign and sin are folded
    into a host-precomputed tan table (cos[j] == cos[partner(j)]), so
    rope(q) = qc + shuffle(qc) * tanb.
  - Scores S^T = K_h^T.T @ Q_h^T per (head, 128-token tile) -- 4x redundant
    (full 128x128 instead of 4 diagonal 32x32 blocks) but instruction-
    efficient. exp on ScalarE (scale + key mask-bias folded in, no max
    subtraction -- scores are bounded), multiplicative block-diagonal mask,
    row sums via a ones-vector PE matmul, normalize P before PV.
  - PV produces A^T directly in feature-partition layout; out-proj consumes
    it and lands token-partition; residual add + store.

ln_gamma is folded into W_qkv on the host; ln_beta (zero in setup_inputs) is
applied to XT as a per-partition bias pass only when nonzero.
"""

import sys
import numpy as np
from contextlib import ExitStack

sys.path.insert(0, "/opt/trn_rl_repo")

D_MODEL = 1024
N_HEADS = 16
D_HEAD = 64
CHUNK = 32
LN_EPS = 1e-5
ROPE_BASE = 10000.0

N_CORES = 8
BLK = 512          # tokens per pipeline block
SUB = 128          # tokens per partition tile
NSUB = BLK // SUB  # 4
ND = D_MODEL // 128  # 8 feature tiles


def build_program(T, with_beta=False, stop_stage=None, repeats=1):
    """Build the per-core Bass program for a T-token shard.

    stop_stage (debug): one of None/'ln'/'xt'/'qk'/'v'/'rope'/'attn'/'pv' --
    truncate the pipeline after that stage and dump its output to ys.
    """
    import concourse.bass as bass
    import concourse.tile as tile
    from concourse import bacc, mybir

    dt = mybir.dt
    AF = mybir.ActivationFunctionType
    OP = mybir.AluOpType

    NBLK = T // BLK
    nc = bacc.Bacc("TRN2", target_bir_lowering=False, debug=False,
                   num_devices=N_CORES)

    xs = nc.dram_tensor("xs", [T, D_MODEL], dt.float32, kind="ExternalInput").ap()
    wqk = nc.dram_tensor("wqk", [D_MODEL, 2 * D_MODEL], dt.float16, kind="ExternalInput").ap()
    wv = nc.dram_tensor("wv", [D_MODEL, D_MODEL], dt.float16, kind="ExternalInput").ap()
    wo = nc.dram_tensor("wo", [D_MODEL, D_MODEL], dt.float16, kind="ExternalInput").ap()
    cosb = nc.dram_tensor("cosb", [128, CHUNK], dt.float16, kind="ExternalInput").ap()
    tanb = nc.dram_tensor("tanb", [128, CHUNK], dt.float16, kind="ExternalInput").ap()
    m01 = nc.dram_tensor("m01", [128, 128], dt.float16, kind="ExternalInput").ap()
    kb = nc.dram_tensor("kb", [128, T // 128], dt.float32, kind="ExternalInput").ap()
    beta = None
    if with_beta:
        beta = nc.dram_tensor("beta", [128, ND], dt.float32, kind="ExternalInput").ap()
    ys = nc.dram_tensor("ys", [T, D_MODEL], dt.float32, kind="ExternalOutput").ap()

    with tile.TileContext(nc) as tc, ExitStack() as ctx:
        const = ctx.enter_context(tc.tile_pool(name="const", bufs=1))
        # ---- constants ----
        wqk_sb = const.tile([128, ND, 2 * D_MODEL], dt.float16, tag="wqk")
        nc.sync.dma_start(wqk_sb[:], wqk.rearrange("(a p) e -> p a e", p=128))
        wv_sb = const.tile([128, ND, D_MODEL], dt.float16, tag="wv")
        nc.sync.dma_start(wv_sb[:], wv.rearrange("(a p) e -> p a e", p=128))
        wo_sb = const.tile([128, ND, D_MODEL], dt.float16, tag="wo")
        nc.sync.dma_start(wo_sb[:], wo.rearrange("(a p) e -> p a e", p=128))
        cos_sb = const.tile([128, CHUNK], dt.float16, tag="cos")
        nc.sync.dma_start(cos_sb[:], cosb)
        tan_sb = const.tile([128, CHUNK], dt.float16, tag="tan")
        nc.sync.dma_start(tan_sb[:], tanb)
        m01_sb = const.tile([128, 128], dt.float16, tag="m01")
        nc.sync.dma_start(m01_sb[:], m01)
        kb_sb = const.tile([128, T // 128], dt.float32, tag="kb")
        nc.sync.dma_start(kb_sb[:], kb)
        ones_sb = const.tile([128, 1], dt.float16, tag="ones")
        nc.gpsimd.memset(ones_sb[:], 1.0)
        eps_sb = const.tile([128, 1], dt.float32, tag="eps")
        nc.gpsimd.memset(eps_sb[:], LN_EPS)
        onesr_sb = const.tile([1, 128], dt.float16, tag="onesr")
        nc.gpsimd.memset(onesr_sb[:], 1.0)
        beta_sb = None
        if with_beta:
            beta_sb = const.tile([128, ND], dt.float32, tag="beta")
            nc.sync.dma_start(beta_sb[:], beta)

        # broadcast views over a 512-wide free dim
        cos_bc = cos_sb[:].unsqueeze(1).to_broadcast((128, BLK // CHUNK, CHUNK))
        tan_bc = tan_sb[:].unsqueeze(1).to_broadcast((128, BLK // CHUNK, CHUNK))
        m01_bc = m01_sb[:].unsqueeze(1).to_broadcast((128, NSUB, 128))

        # ---- pools ----
        xp = ctx.enter_context(tc.tile_pool(name="xp", bufs=2))
        xnp = ctx.enter_context(tc.tile_pool(name="xnp", bufs=4))
        stp = ctx.enter_context(tc.tile_pool(name="stp", bufs=4))
        scp = ctx.enter_context(tc.tile_pool(name="scp", bufs=2))
        xtp = ctx.enter_context(tc.tile_pool(name="xtp", bufs=2))
        qcp = ctx.enter_context(tc.tile_pool(name="qcp", bufs=2))
        qsp = ctx.enter_context(tc.tile_pool(name="qsp", bufs=1))
        vp = ctx.enter_context(tc.tile_pool(name="vp", bufs=4))
        pep = ctx.enter_context(tc.tile_pool(name="pep", bufs=3))
        pmp = ctx.enter_context(tc.tile_pool(name="pmp", bufs=4))
        pnp = ctx.enter_context(tc.tile_pool(name="pnp", bufs=4))
        rcp = ctx.enter_context(tc.tile_pool(name="rcp", bufs=4))
        asp = ctx.enter_context(tc.tile_pool(name="asp", bufs=3))
        xrp = ctx.enter_context(tc.tile_pool(name="xrp", bufs=2))
        yp = ctx.enter_context(tc.tile_pool(name="yp", bufs=2))

        psA = ctx.enter_context(tc.tile_pool(name="psA", bufs=3, space="PSUM"))
        psS = ctx.enter_context(tc.tile_pool(name="psS", bufs=1, space="PSUM"))
        psB = ctx.enter_context(tc.tile_pool(name="psB", bufs=2, space="PSUM"))

        for b in range(NBLK * repeats):
            b = b % NBLK
            t0 = b * BLK
            # ---------- LayerNorm (token-partition) ----------
            xn_tiles = []
            for tt in range(NSUB):
                xt = xp.tile([128, D_MODEL], dt.float32, tag="x")
                nc.sync.dma_start(xt[:], xs[t0 + tt * SUB: t0 + (tt + 1) * SUB, :])
                s1 = stp.tile([128, 1], dt.float32, tag="s1")
                nc.vector.reduce_sum(s1[:], xt[:], axis=mybir.AxisListType.X)
                scr = scp.tile([128, D_MODEL], dt.float16, tag="scr")
                s2 = stp.tile([128, 1], dt.float32, tag="s2")
                nc.scalar.activation(scr[:], xt[:], AF.Square, accum_out=s2[:])
                mu = stp.tile([128, 1], dt.float32, tag="mu")
                nc.vector.tensor_scalar_mul(mu[:], s1[:], 1.0 / D_MODEL)
                mu2 = stp.tile([128, 1], dt.float32, tag="mu2")
                nc.vector.tensor_tensor(mu2[:], mu[:], mu[:], op=OP.mult)
                var = stp.tile([128, 1], dt.float32, tag="var")
                nc.vector.scalar_tensor_tensor(var[:], s2[:], 1.0 / D_MODEL,
                                               mu2[:], op0=OP.mult,
                                               op1=OP.subtract)
                std = stp.tile([128, 1], dt.float32, tag="sd")
                nc.scalar.activation(std[:], var[:], AF.Sqrt, bias=eps_sb[:])
                rstd = stp.tile([128, 1], dt.float32, tag="rs")
                nc.vector.reciprocal(rstd[:], std[:])
                xn = xnp.tile([128, D_MODEL], dt.float16, tag="xn")
                nc.vector.tensor_scalar(xn[:], xt[:], mu[:], rstd[:],
                                        op0=OP.subtract, op1=OP.mult)
                xn_tiles.append(xn)

            if stop_stage == 'ln':
                dbg = yp.tile([128, D_MODEL], dt.float32, tag="y")
                nc.vector.tensor_copy(dbg[:], xn_tiles[0][:])
                nc.sync.dma_start(ys[t0:t0 + SUB, :], dbg[:])
                continue

            # ---------- transpose to feature-partition ----------
            XT = xtp.tile([128, ND, BLK], dt.float16, tag="xt")
            for tt in range(NSUB):
                for dtile in range(ND):
                    eng = nc.sync if (tt * ND + dtile) % 2 == 0 else nc.scalar
                    eng.dma_start_transpose(
                        XT[:, dtile, tt * SUB:(tt + 1) * SUB],
                        xn_tiles[tt][:, dtile * 128:(dtile + 1) * 128])
            if with_beta:
                for dtile in range(ND):
                    nc.scalar.activation(XT[:, dtile, :], XT[:, dtile, :],
                                         AF.Identity, bias=beta_sb[:, dtile:dtile + 1])

            if stop_stage == 'xt':
                dbg = yp.tile([128, D_MODEL], dt.float32, tag="y")
                nc.vector.tensor_copy(dbg[:, 0:512], XT[:, 0, :])
                nc.sync.dma_start(ys[t0:t0 + SUB, :], dbg[:])
                continue

            # ---------- qk projection (feature-partition out) + cos fuse ----
            q_all = qcp.tile([128, ND, BLK], dt.float16, tag="qall")
            k_all = qcp.tile([128, ND, BLK], dt.float16, tag="kall")
            for et in range(16):
                ps = psA.tile([128, BLK], dt.float32, tag="ps512")
                for dtile in range(ND):
                    nc.tensor.matmul(ps[:],
                                     wqk_sb[:, dtile, et * 128:(et + 1) * 128],
                                     XT[:, dtile, :],
                                     start=(dtile == 0), stop=(dtile == ND - 1))
                tgt = q_all if et < 8 else k_all
                nc.vector.tensor_tensor(
                    tgt[:, et % 8, :].rearrange("p (a c) -> p a c", c=CHUNK),
                    ps[:].rearrange("p (a c) -> p a c", c=CHUNK),
                    cos_bc, op=OP.mult)

            if stop_stage == 'qk':
                dbg = yp.tile([128, D_MODEL], dt.float32, tag="y")
                nc.vector.tensor_copy(dbg[:, 0:512], q_all[:, 0, :])
                nc.sync.dma_start(ys[t0:t0 + SUB, :], dbg[:])
                continue

            # ---------- v projection (token-partition out) ----------
            v_tiles = []
            for tt in range(NSUB):
                vt = vp.tile([128, D_MODEL], dt.float16, tag="v")
                for n in range(2):
                    ps = psA.tile([128, BLK], dt.float32, tag="ps512")
                    for dtile in range(ND):
                        nc.tensor.matmul(ps[:],
                                         XT[:, dtile, tt * SUB:(tt + 1) * SUB],
                                         wv_sb[:, dtile, n * 512:(n + 1) * 512],
                                         start=(dtile == 0), stop=(dtile == ND - 1))
                    nc.scalar.copy(vt[:, n * 512:(n + 1) * 512], ps[:])
                v_tiles.append(vt)

            if stop_stage == 'v':
                dbg = yp.tile([128, D_MODEL], dt.float32, tag="y")
                nc.vector.tensor_copy(dbg[:], v_tiles[0][:])
                nc.sync.dma_start(ys[t0:t0 + SUB, :], dbg[:])
                continue

            # ---------- rope: shuffle (+-32 partitions) and combine ----------
            tan_bc_big = tan_sb[:].unsqueeze(1).to_broadcast(
                (128, ND * BLK // CHUNK, CHUNK))
            for src_t, eng in ((q_all, nc.sync), (k_all, nc.scalar)):
                qs = qsp.tile([128, ND, BLK], dt.float16, tag="qs")
                for (o, i) in ((0, 32), (32, 0), (64, 96), (96, 64)):
                    eng.dma_start(qs[o:o + 32, :, :], src_t[i:i + 32, :, :])
                nc.gpsimd.tensor_tensor(
                    qs[:].rearrange("p a (b c) -> p (a b) c", c=CHUNK),
                    qs[:].rearrange("p a (b c) -> p (a b) c", c=CHUNK),
                    tan_bc_big, op=OP.mult)
                nc.vector.tensor_tensor(src_t[:], src_t[:], qs[:], op=OP.add)

            if stop_stage == 'rope':
                dbg = yp.tile([128, D_MODEL], dt.float32, tag="y")
                nc.vector.tensor_copy(dbg[:, 0:512], q_all[:, 0, :])
                nc.sync.dma_start(ys[t0:t0 + SUB, :], dbg[:])
                continue

            # ---------- attention per 128-token tile ----------
            for tt in range(NSUB):
                pn_tiles = []
                for hg in range(4):
                    # heads in group hg share partition parity so all four
                    # matmuls into this PSUM bank use the same row-group base
                    # (mixed-base concurrent PE writes to one bank fault HW)
                    heads = [(hg // 2) * 8 + (hg % 2) + 2 * hh for hh in range(4)]
                    sps = psA.tile([128, BLK], dt.float32, tag="ps512")
                    for hh, h in enumerate(heads):
                        et, po = h // 2, (h % 2) * 64
                        ksl = k_all[po:po + 64, et, tt * SUB:(tt + 1) * SUB]
                        qsl = q_all[po:po + 64, et, tt * SUB:(tt + 1) * SUB]
                        nc.tensor.matmul(sps[:, hh * 128:(hh + 1) * 128],
                                         ksl, qsl, start=True, stop=True)
                    pexp = pep.tile([128, BLK], dt.float16, tag="pe")
                    bidx = (t0 // SUB) + tt
                    if stop_stage == 'attn0':
                        nc.vector.tensor_copy(pexp[:], sps[:])
                        pn_tiles.append(pexp)
                        continue
                    nc.scalar.activation(pexp[:], sps[:], AF.Exp,
                                         scale=float(D_HEAD) ** -0.5,
                                         bias=kb_sb[:, bidx:bidx + 1])
                    if stop_stage == 'attn1':
                        pn_tiles.append(pexp)
                        continue
                    pm = pmp.tile([128, BLK], dt.float16, tag="pm")
                    nc.gpsimd.tensor_tensor(
                        pm[:].rearrange("p (a c) -> p a c", c=128),
                        pexp[:].rearrange("p (a c) -> p a c", c=128),
                        m01_bc, op=OP.mult)
                    if stop_stage == 'attn2':
                        pn_tiles.append(pm)
                        continue
                    sums = psS.tile([1, BLK], dt.float32, tag="sum")
                    nc.tensor.matmul(sums[:], ones_sb[:], pm[:],
                                     start=True, stop=True)
                    rc = rcp.tile([1, BLK], dt.float16, tag="rc")
                    with nc.allow_low_precision(reason="softmax denominators are O(1..1e4); fp16 recip is plenty"):
                        nc.vector.reciprocal(rc[:], sums[:])
                    if stop_stage == 'attn4':
                        pn_tiles.append(pm)
                        continue
                    rcb = psA.tile([128, BLK], dt.float32, tag="ps512")
                    nc.tensor.matmul(rcb[:], onesr_sb[:], rc[:],
                                     start=True, stop=True)
                    pn = pnp.tile([128, BLK], dt.float16, tag="pn")
                    nc.vector.tensor_tensor(pn[:], pm[:], rcb[:], op=OP.mult)
                    pn_tiles.append(pn)

                if stop_stage in ('attn', 'attn0', 'attn1', 'attn2', 'attn4'):
                    dbg = yp.tile([128, D_MODEL], dt.float32, tag="y")
                    nc.vector.tensor_copy(dbg[:, 0:512], pn_tiles[0][:])
                    nc.sync.dma_start(ys[t0 + tt * SUB:t0 + (tt + 1) * SUB, :], dbg[:])
                    continue

                # ---------- PV: A^T in feature-partition ----------
                # A^T via PV. Even heads accumulate in ape (partition base 0),
                # odd heads in apo (base 64) -- a PSUM bank must only ever see
                # one partition base from the PE.
                ape = psB.tile([128, D_MODEL], dt.float32, tag="big")
                apo = psB.tile([128, D_MODEL], dt.float32, tag="big")
                for h in range(N_HEADS):
                    g = 2 * (h // 8) + (h % 2)
                    col = (h % 8) // 2
                    po = (h % 2) * 64
                    tgt = apo if (h % 2) else ape
                    # head h -> window (h//2) in its parity tile
                    dp = h // 2
                    nc.tensor.matmul(
                        tgt[po:po + 64, dp * 128:(dp + 1) * 128],
                        v_tiles[tt][:, h * D_HEAD:(h + 1) * D_HEAD],
                        pn_tiles[g][:, col * 128:(col + 1) * 128],
                        start=True, stop=True)
                asb = asp.tile([128, ND, SUB], dt.float16, tag="a")
                for dp in range(ND):
                    se = ape[0:64, dp * 128:(dp + 1) * 128]
                    so = apo[64:128, dp * 128:(dp + 1) * 128]
                    if dp % 2 == 0:
                        nc.scalar.copy(asb[0:64, dp, :], se)
                        nc.vector.tensor_copy(asb[64:128, dp, :], so)
                    else:
                        nc.vector.tensor_copy(asb[0:64, dp, :], se)
                        nc.scalar.copy(asb[64:128, dp, :], so)

                if stop_stage == 'pv':
                    dbg = yp.tile([128, D_MODEL], dt.float32, tag="y")
                    nc.vector.tensor_copy(dbg[:], asb[:].rearrange("p a c -> p (a c)"))
                    nc.sync.dma_start(ys[t0 + tt * SUB:t0 + (tt + 1) * SUB, :], dbg[:])
                    continue

                # ---------- out projection + residual ----------
                ops = psB.tile([128, D_MODEL], dt.float32, tag="big")
                for n in range(2):
                    for dp in range(ND):
                        nc.tensor.matmul(ops[:, n * 512:(n + 1) * 512],
                                         asb[:, dp, :],
                                         wo_sb[:, dp, n * 512:(n + 1) * 512],
                                         start=(dp == 0), stop=(dp == ND - 1))
                xr = xrp.tile([128, D_MODEL], dt.float32, tag="xr")
                rows = slice(t0 + tt * SUB, t0 + (tt + 1) * SUB)
                nc.sync.dma_start(xr[:], xs[rows, :])
                y = yp.tile([128, D_MODEL], dt.float32, tag="y")
                nc.vector.tensor_tensor(y[:], ops[:], xr[:], op=OP.add)
                nc.sync.dma_start(ys[rows, :], y[:])

    nc.compile()
    return nc


def host_inputs(x, mask, ln_gamma, ln_beta, W_qkv, W_out, T):
    """Prepare per-core input maps. x: (B, L, D) fp32."""
    B, L, D = x.shape
    tokens = B * L
    n_cores = tokens // T
    W_eff = (W_qkv * ln_gamma[None, :]).astype(np.float32)
    wqk_h = np.ascontiguousarray(W_eff[0:2 * D].T).astype(np.float16)
    wv_h = np.ascontiguousarray(W_eff[2 * D:3 * D].T).astype(np.float16)
    wo_h = np.ascontiguousarray(W_out.T).astype(np.float16)

    inv_freq = 1.0 / (ROPE_BASE ** (np.arange(0, D_HEAD, 2) / D_HEAD))  # (32,)
    p = np.arange(128)
    j = p % D_HEAD
    idx = j % 32
    sign = np.where(j < 32, -1.0, 1.0)
    t = np.arange(CHUNK)
    ang = t[None, :] * inv_freq[idx][:, None]          # (128, 32)
    cos_h = np.cos(ang).astype(np.float16)
    tan_h = (sign[:, None] * np.tan(ang)).astype(np.float16)

    ii = np.arange(128)
    m01_h = (ii[:, None] // CHUNK == ii[None, :] // CHUNK).astype(np.float16)

    xs_flat = np.ascontiguousarray(x.reshape(tokens, D).astype(np.float32))
    mask_flat = mask.reshape(tokens).astype(np.float32)
    kbias = np.where(mask_flat == 0, -30000.0, 0.0).astype(np.float32)

    shared = {"wqk": wqk_h, "wv": wv_h, "wo": wo_h,
              "cosb": cos_h, "tanb": tan_h, "m01": m01_h}
    with_beta = bool(np.any(ln_beta != 0))
    if with_beta:
        shared["beta"] = np.ascontiguousarray(
            ln_beta.reshape(ND, 128).T).astype(np.float32)

    in_maps = []
    for c in range(n_cores):
        sl = slice(c * T, (c + 1) * T)
        kb_c = np.ascontiguousarray(
            kbias[sl].reshape(T // 128, 128).T).astype(np.float32)
        m = dict(shared)
        m["xs"] = xs_flat[sl]
        m["kb"] = kb_c
        in_maps.append(m)
    return in_maps, with_beta


_PROGRAM_CACHE = {}


def kernel(x, mask, ln_gamma, ln_beta, W_qkv, W_out):
    from concourse import bass_utils

    x = np.asarray(x, dtype=np.float32)
    mask = np.asarray(mask, dtype=np.float32)
    ln_gamma = np.asarray(ln_gamma, dtype=np.float32)
    ln_beta = np.asarray(ln_beta, dtype=np.float32)
    W_qkv = np.asarray(W_qkv, dtype=np.float32)
    W_out = np.asarray(W_out, dtype=np.float32)

    B, L, D = x.shape
    T = (B * L) // N_CORES
    in_maps, with_beta = host_inputs(x, mask, ln_gamma, ln_beta, W_qkv, W_out, T)

    key = (T, with_beta)
    if key not in _PROGRAM_CACHE:
        _PROGRAM_CACHE[key] = build_program(T, with_beta=with_beta)
    nc = _PROGRAM_CACHE[key]

    res = bass_utils.run_bass_kernel_spmd(nc, in_maps, core_ids=list(range(N_CORES)))
    ys = np.concatenate([res.results[c]["ys"] for c in range(N_CORES)], axis=0)
    return ys.reshape(B, L, D).astype(np.float32)


if __name__ == "__main__":
    rng = np.random.default_rng(0)
    B, L = 4, 4096
    x = rng.standard_normal((B, L, D_MODEL), dtype=np.float32)
    mask = np.ones((B, L), dtype=np.float32)
    g = np.ones(D_MODEL, dtype=np.float32)
    be = np.zeros(D_MODEL, dtype=np.float32)
    Wq = (rng.standard_normal((3 * D_MODEL, D_MODEL)) * 0.02).astype(np.float32)
    Wo = (rng.standard_normal((D_MODEL, D_MODEL)) * 0.02).astype(np.float32)
    y = kernel(x, mask, g, be, Wq, Wo)
    print("kernel output:", y.shape, y.dtype)



# revision 18
# speedup vs baseline: 135.5032x; 135.5032x over previous
"""Trainium2 Bass kernel for nn_LocalSmoother (LN -> QKV -> RoPE -> 32-token
block-diagonal attention -> out-proj -> residual).

Sharding: B*L = 16384 tokens split evenly across 8 cores (2048 tokens each,
64 chunks of 32). Attention is block-diagonal over 32-token chunks, so shards
are fully independent (pure SPMD, no collectives). Weights are replicated.

Per-core layout strategy:
  - LayerNorm in token-partition layout, normalize via one fused
    tensor_scalar, output fp16.
  - xn transposed to feature-partition layout (XT) via DMA xbar transpose.
  - QKV as fp16 PE matmuls producing q^T/k^T (feature-partition) and V
    (token-partition).
  - RoPE: qc = q * cos fused into the PSUM->SBUF move; rotate-half is a
    +-32-partition shuffle done with SBUF->SBUF DMA; sign and sin are folded
    into a host-precomputed tan table (cos[j] == cos[partner(j)]), so
    rope(q) = qc + shuffle(qc) * tanb.
  - Scores S^T = K_h^T.T @ Q_h^T per (head, 128-token tile) -- 4x redundant
    (full 128x128 instead of 4 diagonal 32x32 blocks) but instruction-
    efficient. The block-diagonal mask is folded INTO the scores as a rank-5
    additive bias (-30000 off diagonal blocks) supplied by one extra PE
    matmul per score tile (mk^T @ mq), so exp() alone produces masked P.
  - exp on ScalarE (scale + key mask-bias folded in, no max subtraction --
    scores are bounded), row sums via a ones-vector PE matmul, normalize P
    before PV.
  - PV produces A^T directly in feature-partition layout; out-proj consumes
    it and lands token-partition; residual add + store (fp16, widened on
    host).

ln_gamma is folded into W_qkv on the host; ln_beta (zero in setup_inputs) is
applied to XT as a per-partition bias pass only when nonzero.
"""

import sys
import numpy as np
from contextlib import ExitStack

sys.path.insert(0, "/opt/trn_rl_repo")

D_MODEL = 1024
N_HEADS = 16
D_HEAD = 64
CHUNK = 32
LN_EPS = 1e-5
ROPE_BASE = 10000.0
MASK_BIG = 30000.0

N_CORES = 8
BLK = 512          # tokens per pipeline block
SUB = 128          # tokens per partition tile
NSUB = BLK // SUB  # 4
ND = D_MODEL // 128  # 8 feature tiles


def build_program(T, with_beta=False, repeats=1):
    """Build the per-core Bass program for a T-token shard."""
    import concourse.bass as bass
    import concourse.tile as tile
    from concourse import bacc, mybir

    dt = mybir.dt
    AF = mybir.ActivationFunctionType
    OP = mybir.AluOpType

    NBLK = T // BLK
    nc = bacc.Bacc("TRN2", target_bir_lowering=False, debug=False,
                   num_devices=N_CORES)

    xs = nc.dram_tensor("xs", [T, D_MODEL], dt.float32, kind="ExternalInput").ap()
    wqk = nc.dram_tensor("wqk", [D_MODEL, 2 * D_MODEL], dt.float16, kind="ExternalInput").ap()
    wv = nc.dram_tensor("wv", [D_MODEL, D_MODEL], dt.float16, kind="ExternalInput").ap()
    wo = nc.dram_tensor("wo", [D_MODEL, D_MODEL], dt.float16, kind="ExternalInput").ap()
    cosb = nc.dram_tensor("cosb", [128, CHUNK], dt.float16, kind="ExternalInput").ap()
    tanb = nc.dram_tensor("tanb", [128, CHUNK], dt.float16, kind="ExternalInput").ap()
    mkb = nc.dram_tensor("mkb", [128, 128], dt.float16, kind="ExternalInput").ap()
    mqb = nc.dram_tensor("mqb", [128, BLK], dt.float16, kind="ExternalInput").ap()
    kb = nc.dram_tensor("kb", [128, T // 128], dt.float32, kind="ExternalInput").ap()
    beta = None
    if with_beta:
        beta = nc.dram_tensor("beta", [128, ND], dt.float32, kind="ExternalInput").ap()
    ys = nc.dram_tensor("ys", [T, D_MODEL], dt.float16, kind="ExternalOutput").ap()

    with tile.TileContext(nc) as tc, ExitStack() as ctx:
        const = ctx.enter_context(tc.tile_pool(name="const", bufs=1))
        # ---- constants ----
        # weights go on the scalar HW queue so block 0's x loads (sync queue)
        # aren't stuck behind 8.4MB of weight traffic
        wqk_sb = const.tile([128, ND, 2 * D_MODEL], dt.float16, tag="wqk")
        nc.scalar.dma_start(wqk_sb[:], wqk.rearrange("(a p) e -> p a e", p=128))
        wv_sb = const.tile([128, ND, D_MODEL], dt.float16, tag="wv")
        nc.scalar.dma_start(wv_sb[:], wv.rearrange("(a p) e -> p a e", p=128))
        wo_sb = const.tile([128, ND, D_MODEL], dt.float16, tag="wo")
        nc.scalar.dma_start(wo_sb[:], wo.rearrange("(a p) e -> p a e", p=128))
        cos_sb = const.tile([128, CHUNK], dt.float16, tag="cos")
        nc.sync.dma_start(cos_sb[:], cosb)
        tan_sb = const.tile([128, CHUNK], dt.float16, tag="tan")
        nc.sync.dma_start(tan_sb[:], tanb)
        mk_sb = const.tile([128, 128], dt.float16, tag="mk")
        nc.sync.dma_start(mk_sb[:], mkb)
        mq_sb = const.tile([128, BLK], dt.float16, tag="mq")
        nc.sync.dma_start(mq_sb[:], mqb)
        kb_sb = const.tile([128, T // 128], dt.float32, tag="kb")
        nc.sync.dma_start(kb_sb[:], kb)
        ones_sb = const.tile([128, 1], dt.float16, tag="ones")
        nc.gpsimd.memset(ones_sb[:], 1.0)
        eps_sb = const.tile([128, 1], dt.float32, tag="eps")
        nc.gpsimd.memset(eps_sb[:], LN_EPS)
        onesr_sb = const.tile([1, 128], dt.float16, tag="onesr")
        nc.gpsimd.memset(onesr_sb[:], 1.0)
        # shared scratch for the LN Square activation's (unused) main output;
        # all writers are on the in-order Act queue, so reuse is safe
        sqscr_sb = const.tile([128, D_MODEL], dt.float16, tag="sqscr")
        beta_sb = None
        if with_beta:
            beta_sb = const.tile([128, ND], dt.float32, tag="beta")
            nc.sync.dma_start(beta_sb[:], beta)

        # broadcast views over a 512-wide free dim
        cos_bc = cos_sb[:].unsqueeze(1).to_broadcast((128, BLK // CHUNK, CHUNK))

        # ---- pools ----
        xp = ctx.enter_context(tc.tile_pool(name="xp", bufs=4))
        xnp = ctx.enter_context(tc.tile_pool(name="xnp", bufs=8))
        stp = ctx.enter_context(tc.tile_pool(name="stp", bufs=8))
        xtp = ctx.enter_context(tc.tile_pool(name="xtp", bufs=2))
        qcp = ctx.enter_context(tc.tile_pool(name="qcp", bufs=2))
        qsp = ctx.enter_context(tc.tile_pool(name="qsp", bufs=2))
        vp = ctx.enter_context(tc.tile_pool(name="vp", bufs=4))
        pep = ctx.enter_context(tc.tile_pool(name="pep", bufs=3))
        pnp = ctx.enter_context(tc.tile_pool(name="pnp", bufs=4))
        rcp = ctx.enter_context(tc.tile_pool(name="rcp", bufs=4))
        asp = ctx.enter_context(tc.tile_pool(name="asp", bufs=3))
        xrp = ctx.enter_context(tc.tile_pool(name="xrp", bufs=2))
        yp = ctx.enter_context(tc.tile_pool(name="yp", bufs=2))

        # P1: single 4-deep rotation for every [128,512] PSUM use (qk ps, v ps,
        # score tiles, merged rowsum+recip-broadcast tiles). psB: the two
        # double-bank tiles (PV accumulators ape/apo, out-proj).
        psA = ctx.enter_context(tc.tile_pool(name="psA", bufs=4, space="PSUM"))
        psB = ctx.enter_context(tc.tile_pool(name="psB", bufs=2, space="PSUM"))

        # deferred out-proj emission (software pipelining): emit the previous
        # tile's out-proj matmuls only after the next tile's first score
        # matmuls so the PE queue never sits on an asb/exp/recip dependency.
        pending_out = []

        def flush_pending():
            while pending_out:
                pending_out.pop(0)()

        for b in range(NBLK * repeats):
            b = b % NBLK
            t0 = b * BLK
            # ---------- LayerNorm (token-partition) ----------
            xn_tiles = []
            for tt in range(NSUB):
                xt = xp.tile([128, D_MODEL], dt.float32, tag="x")
                nc.sync.dma_start(xt[:], xs[t0 + tt * SUB: t0 + (tt + 1) * SUB, :])
                s1 = stp.tile([128, 1], dt.float32, tag="s1")
                nc.vector.reduce_sum(s1[:], xt[:], axis=mybir.AxisListType.X)
                s2 = stp.tile([128, 1], dt.float32, tag="s2")
                nc.scalar.activation(sqscr_sb[:], xt[:], AF.Square, accum_out=s2[:])
                mu = stp.tile([128, 1], dt.float32, tag="mu")
                nc.vector.tensor_scalar_mul(mu[:], s1[:], 1.0 / D_MODEL)
                mu2 = stp.tile([128, 1], dt.float32, tag="mu2")
                nc.vector.tensor_tensor(mu2[:], mu[:], mu[:], op=OP.mult)
                var = stp.tile([128, 1], dt.float32, tag="var")
                nc.vector.scalar_tensor_tensor(var[:], s2[:], 1.0 / D_MODEL,
                                               mu2[:], op0=OP.mult,
                                               op1=OP.subtract)
                std = stp.tile([128, 1], dt.float32, tag="sd")
                nc.scalar.activation(std[:], var[:], AF.Sqrt, bias=eps_sb[:])
                rstd = stp.tile([128, 1], dt.float32, tag="rs")
                nc.vector.reciprocal(rstd[:], std[:])
                xn = xnp.tile([128, D_MODEL], dt.float16, tag="xn")
                nc.vector.tensor_scalar(xn[:], xt[:], mu[:], rstd[:],
                                        op0=OP.subtract, op1=OP.mult)
                xn_tiles.append(xn)

            # ---------- transpose to feature-partition ----------
            XT = xtp.tile([128, ND, BLK], dt.float16, tag="xt")
            for tt in range(NSUB):
                for dtile in range(ND):
                    eng = nc.sync if (tt * ND + dtile) % 2 == 0 else nc.scalar
                    eng.dma_start_transpose(
                        XT[:, dtile, tt * SUB:(tt + 1) * SUB],
                        xn_tiles[tt][:, dtile * 128:(dtile + 1) * 128])
            if with_beta:
                for dtile in range(ND):
                    nc.scalar.activation(XT[:, dtile, :], XT[:, dtile, :],
                                         AF.Identity, bias=beta_sb[:, dtile:dtile + 1])

            # ---------- qk projection (feature-partition out) + cos fuse ----
            q_all = qcp.tile([128, ND, BLK], dt.float16, tag="qall")
            k_all = qcp.tile([128, ND, BLK], dt.float16, tag="kall")
            tan_bc_half = tan_sb[:].unsqueeze(1).to_broadcast(
                (128, (ND // 2) * BLK // CHUNK, CHUNK))

            def rope_half(src_t, half, eng_dma, eng_mul, eng_add):
                """rotate-half shuffle + tan-mult + add on et tiles
                [half*4, half*4+4) -- pipelined at half-block granularity so
                scores never wait on one whole-block rope chain."""
                dsl = slice(half * (ND // 2), (half + 1) * (ND // 2))
                qs = qsp.tile([128, ND // 2, BLK], dt.float16, tag="qsh")
                for (o, i) in ((0, 32), (32, 0), (64, 96), (96, 64)):
                    eng_dma.dma_start(qs[o:o + 32, :, :], src_t[i:i + 32, dsl, :])
                eng_mul.tensor_tensor(
                    qs[:].rearrange("p a (b c) -> p (a b) c", c=CHUNK),
                    qs[:].rearrange("p a (b c) -> p (a b) c", c=CHUNK),
                    tan_bc_half, op=OP.mult)
                eng_add.tensor_tensor(src_t[:, dsl, :], src_t[:, dsl, :],
                                      qs[:], op=OP.add)

            for et in range(16):
                ps = psA.tile([128, BLK], dt.float32, tag="ps512")
                for dtile in range(ND):
                    nc.tensor.matmul(ps[:],
                                     wqk_sb[:, dtile, et * 128:(et + 1) * 128],
                                     XT[:, dtile, :],
                                     start=(dtile == 0), stop=(dtile == ND - 1))
                if et == 1:
                    flush_pending()
                tgt = q_all if et < 8 else k_all
                nc.vector.tensor_tensor(
                    tgt[:, et % 8, :].rearrange("p (a c) -> p a c", c=CHUNK),
                    ps[:].rearrange("p (a c) -> p a c", c=CHUNK),
                    cos_bc, op=OP.mult)
                if et == 3:
                    rope_half(q_all, 0, nc.sync, nc.gpsimd, nc.gpsimd)
                elif et == 7:
                    rope_half(q_all, 1, nc.scalar, nc.gpsimd, nc.gpsimd)
                elif et == 11:
                    rope_half(k_all, 0, nc.sync, nc.gpsimd, nc.vector)
                elif et == 15:
                    rope_half(k_all, 1, nc.scalar, nc.gpsimd, nc.vector)

            # ---------- v projection (token-partition out) ----------
            v_tiles = []
            for tt in range(NSUB):
                vt = vp.tile([128, D_MODEL], dt.float16, tag="v")
                for n in range(2):
                    ps = psA.tile([128, BLK], dt.float32, tag="ps512")
                    for dtile in range(ND):
                        nc.tensor.matmul(ps[:],
                                         XT[:, dtile, tt * SUB:(tt + 1) * SUB],
                                         wv_sb[:, dtile, n * 512:(n + 1) * 512],
                                         start=(dtile == 0), stop=(dtile == ND - 1))
                    nc.scalar.copy(vt[:, n * 512:(n + 1) * 512], ps[:])
                v_tiles.append(vt)

            # ---------- attention, software-pipelined over head groups ------
            # stages per group g: sc (mask+scores matmuls, exp on Act),
            # su (rowsum matmul into row 0 of R, reciprocal on DVE),
            # rc (recip broadcast matmul over all of R, pn mult on DVE),
            # pv (4 PV matmuls). Emission order interleaves stages of
            # neighbouring groups so the in-order PE queue never waits on
            # Act/DVE latency.
            for tt in range(NSUB):
                bidx = (t0 // SUB) + tt
                pex_t, R_t, rc_t, pn_t = {}, {}, {}, {}
                sps_t = {}
                ap_t = {}

                def sc(g, tt=tt):
                    heads = [(g // 2) * 8 + (g % 2) + 2 * hh for hh in range(4)]
                    pog = (g % 2) * 64
                    sps = psA.tile([128, BLK], dt.float32, tag="ps512")
                    sps_t[g] = sps
                    # rank-5 block-diagonal mask bias (-MASK_BIG off diagonal
                    # 32-blocks), base-matched to this group's PE row parity
                    # (a PSUM bank must only see one row-group base)
                    nc.tensor.matmul(sps[:], mk_sb[pog:pog + 5, :],
                                     mq_sb[pog:pog + 5, :],
                                     start=True, stop=False)
                    for hh, h in enumerate(heads):
                        et, po = h // 2, (h % 2) * 64
                        ksl = k_all[po:po + 64, et, tt * SUB:(tt + 1) * SUB]
                        qsl = q_all[po:po + 64, et, tt * SUB:(tt + 1) * SUB]
                        nc.tensor.matmul(sps[:, hh * 128:(hh + 1) * 128],
                                         ksl, qsl, start=False, stop=True)
                    pexp = pep.tile([128, BLK], dt.float16, tag="pe")
                    nc.scalar.activation(pexp[:], sps[:], AF.Exp,
                                         scale=float(D_HEAD) ** -0.5,
                                         bias=kb_sb[:, bidx:bidx + 1])
                    pex_t[g] = pexp

                def su(g):
                    R = psA.tile([128, BLK], dt.float32, tag="ps512")
                    R_t[g] = R
                    nc.tensor.matmul(R[0:1, :], ones_sb[:], pex_t[g][:],
                                     start=True, stop=True)
                    rc = rcp.tile([1, BLK], dt.float16, tag="rc")
                    with nc.allow_low_precision(reason="softmax denominators are O(1..1e4); fp16 recip is plenty"):
                        nc.vector.reciprocal(rc[:], R[0:1, :])
                    rc_t[g] = rc

                def rc(g):
                    nc.tensor.matmul(R_t[g][:], onesr_sb[:], rc_t[g][:],
                                     start=True, stop=True)
                    pn = pnp.tile([128, BLK], dt.float16, tag="pn")
                    nc.vector.tensor_tensor(pn[:], pex_t[g][:], R_t[g][:],
                                            op=OP.mult)
                    pn_t[g] = pn

                def pv(g, tt=tt):
                    if not ap_t:
                        ap_t["e"] = psB.tile([128, D_MODEL], dt.float32,
                                             tag="big", name="ape")
                        ap_t["o"] = psB.tile([128, D_MODEL], dt.float32,
                                             tag="big", name="apo")
                    for h in range(N_HEADS):
                        if 2 * (h // 8) + (h % 2) != g:
                            continue
                        col = (h % 8) // 2
                        po = (h % 2) * 64
                        tgt = ap_t["o"] if (h % 2) else ap_t["e"]
                        dp = h // 2
                        nc.tensor.matmul(
                            tgt[po:po + 64, dp * 128:(dp + 1) * 128],
                            v_tiles[tt][:, h * D_HEAD:(h + 1) * D_HEAD],
                            pn_t[g][:, col * 128:(col + 1) * 128],
                            start=True, stop=True)

                sc(0); sc(1); su(0)
                # previous tile's out-proj lands here: useful PE work that
                # hides this tile's exp/recip latencies, and its psB slot is
                # taken before this tile's PV accumulators claim theirs.
                if pending_out:
                    pending_out.pop(0)()
                sc(2); rc(0); su(1); sc(3); rc(1)
                su(2); pv(0); rc(2); su(3); pv(1); rc(3); pv(2); pv(3)

                asb = asp.tile([128, ND, SUB], dt.float16, tag="a")
                nc.vector.tensor_copy(
                    asb[0:64, :, :],
                    ap_t["e"][0:64, :].rearrange("p (a c) -> p a c", c=SUB))
                nc.scalar.copy(
                    asb[64:128, :, :],
                    ap_t["o"][64:128, :].rearrange("p (a c) -> p a c", c=SUB))

                # residual row loads issued early; out-proj emission deferred
                xr = xrp.tile([128, D_MODEL], dt.float32, tag="xr")
                rows = slice(t0 + tt * SUB, t0 + (tt + 1) * SUB)
                nc.sync.dma_start(xr[:], xs[rows, :])

                def out_proj(asb=asb, xr=xr, rows=rows):
                    ops = psB.tile([128, D_MODEL], dt.float32, tag="big")
                    for n in range(2):
                        for dp in range(ND):
                            nc.tensor.matmul(ops[:, n * 512:(n + 1) * 512],
                                             asb[:, dp, :],
                                             wo_sb[:, dp, n * 512:(n + 1) * 512],
                                             start=(dp == 0), stop=(dp == ND - 1))
                    y = yp.tile([128, D_MODEL], dt.float16, tag="y")
                    nc.vector.tensor_tensor(y[:], ops[:], xr[:], op=OP.add)
                    nc.scalar.dma_start(ys[rows, :], y[:])

                pending_out.append(out_proj)

        flush_pending()

    nc.compile()
    return nc


def host_inputs(x, mask, ln_gamma, ln_beta, W_qkv, W_out, T):
    """Prepare per-core input maps. x: (B, L, D) fp32."""
    B, L, D = x.shape
    tokens = B * L
    n_cores = tokens // T
    W_eff = (W_qkv * ln_gamma[None, :]).astype(np.float32)
    wqk_h = np.ascontiguousarray(W_eff[0:2 * D].T).astype(np.float16)
    wv_h = np.ascontiguousarray(W_eff[2 * D:3 * D].T).astype(np.float16)
    wo_h = np.ascontiguousarray(W_out.T).astype(np.float16)

    inv_freq = 1.0 / (ROPE_BASE ** (np.arange(0, D_HEAD, 2) / D_HEAD))  # (32,)
    p = np.arange(128)
    j = p % D_HEAD
    idx = j % 32
    sign = np.where(j < 32, -1.0, 1.0)
    t = np.arange(CHUNK)
    ang = t[None, :] * inv_freq[idx][:, None]          # (128, 32)
    cos_h = np.cos(ang).astype(np.float16)
    tan_h = (sign[:, None] * np.tan(ang)).astype(np.float16)

    # rank-5 block-diagonal mask factors, duplicated at partition bases 0/64
    # so the mask matmul can match each head group's PE row base.
    # (mk^T @ mq)[i, j] = -MASK_BIG + MASK_BIG * [i//32 == (j%128)//32]
    rB = np.float16(np.sqrt(MASK_BIG))
    mk_h = np.zeros((128, 128), dtype=np.float16)
    mq_h = np.zeros((128, BLK), dtype=np.float16)
    ii = np.arange(128)
    jj = np.arange(BLK)
    for base in (0, 64):
        mk_h[base + 0, :] = rB
        mq_h[base + 0, :] = -rB
        for bq in range(4):
            mk_h[base + 1 + bq, :] = rB * (ii // CHUNK == bq)
            mq_h[base + 1 + bq, :] = rB * ((jj % 128) // CHUNK == bq)

    xs_flat = np.ascontiguousarray(x.reshape(tokens, D).astype(np.float32))
    mask_flat = mask.reshape(tokens).astype(np.float32)
    kbias = np.where(mask_flat == 0, -MASK_BIG, 0.0).astype(np.float32)

    shared = {"wqk": wqk_h, "wv": wv_h, "wo": wo_h,
              "cosb": cos_h, "tanb": tan_h, "mkb": mk_h, "mqb": mq_h}
    with_beta = bool(np.any(ln_beta != 0))
    if with_beta:
        shared["beta"] = np.ascontiguousarray(
            ln_beta.reshape(ND, 128).T).astype(np.float32)

    in_maps = []
    for c in range(n_cores):
        sl = slice(c * T, (c + 1) * T)
        kb_c = np.ascontiguousarray(
            kbias[sl].reshape(T // 128, 128).T).astype(np.float32)
        m = dict(shared)
        m["xs"] = xs_flat[sl]
        m["kb"] = kb_c
        in_maps.append(m)
    return in_maps, with_beta


_PROGRAM_CACHE = {}


def kernel(x, mask, ln_gamma, ln_beta, W_qkv, W_out):
    from concourse import bass_utils

    x = np.asarray(x, dtype=np.float32)
    mask = np.asarray(mask, dtype=np.float32)
    ln_gamma = np.asarray(ln_gamma, dtype=np.float32)
    ln_beta = np.asarray(ln_beta, dtype=np.float32)
    W_qkv = np.asarray(W_qkv, dtype=np.float32)
    W_out = np.asarray(W_out, dtype=np.float32)

    B, L, D = x.shape
    T = (B * L) // N_CORES
    in_maps, with_beta = host_inputs(x, mask, ln_gamma, ln_beta, W_qkv, W_out, T)

    key = (T, with_beta)
    if key not in _PROGRAM_CACHE:
        _PROGRAM_CACHE[key] = build_program(T, with_beta=with_beta)
    nc = _PROGRAM_CACHE[key]

    res = bass_utils.run_bass_kernel_spmd(nc, in_maps, core_ids=list(range(N_CORES)))
    ys = np.concatenate([res.results[c]["ys"] for c in range(N_CORES)], axis=0)
    return ys.reshape(B, L, D).astype(np.float32)


if __name__ == "__main__":
    rng = np.random.default_rng(0)
    B, L = 4, 4096
    x = rng.standard_normal((B, L, D_MODEL), dtype=np.float32)
    mask = np.ones((B, L), dtype=np.float32)
    g = np.ones(D_MODEL, dtype=np.float32)
    be = np.zeros(D_MODEL, dtype=np.float32)
    Wq = (rng.standard_normal((3 * D_MODEL, D_MODEL)) * 0.02).astype(np.float32)
    Wo = (rng.standard_normal((D_MODEL, D_MODEL)) * 0.02).astype(np.float32)
    y = kernel(x, mask, g, be, Wq, Wo)
    print("kernel output:", y.shape, y.dtype)
